# revision 13
# baseline (speedup 1.0000x reference)
"""DCN block kernel for Trainium2 (8 NeuronCores, data-parallel over batch).

Math (per batch b, plane c):
  z   = conv3x3(x, w_off) + b_off                  (64 offset logits)
  s   = sigmoid(z); d = s - 0.5 in (-.5,.5)        (pixel displacement)
  sample at (r - dy, c - dx) bilinear w/ reflect   (|d| < .5 => 3x3 support)
  y   = conv3x3(sampled, w_dcn) + b_dcn

Gather-free sampling via relu-split weights (all DVE ops run in 2x/4x mode):
  A  = relu(sx-.5)  B = relu(.5-sx)   (horizontal taps)
  H_dr = x_dr + A*DL_dr + B*DR_dr     DL = x(c-1)-x(c), DR = x(c+1)-x(c)
  OS = H0 + Av*(Hm-H0) + Bv*(Hp-H0)   (vertical combine)
Reflect at borders: column fixups on DL/DR; row fixups swap the dead Hm/Hp
term at the global top/bottom rows.

Layout: 4 image row-quarters stacked on partition groups [4 x 32ch]; convs
run as 4 concurrent row-tiled matmul streams (tile_position), 9 taps
accumulating in PSUM.

Two schedule tricks:
 - the offset stage for slab i+1 is issued BEFORE the sampling stage of
   slab i, so PE/scalar work overlaps the vector pipeline;
 - slabs are incremental: the 2 sampled halo rows each slab shares with its
   predecessor are copied from the previous OS tile instead of recomputed,
   so conv_off/sigmoid/sampling all run on nr rows, not nr+2.
"""

from contextlib import ExitStack

import ml_dtypes
import numpy as np

import concourse.bacc as bacc
import concourse.bass as bass
import concourse.mybir as mybir
import concourse.tile as tile

BF16 = mybir.dt.bfloat16
F32 = mybir.dt.float32
AF = mybir.ActivationFunctionType
OP = mybir.AluOpType

N_CORES = 8
C = 32          # input/output channels per plane set
OC2 = 64        # offset logits (2 per plane)


class Cfg:
    def __init__(self, H=384, nr=8):
        self.H = H
        self.W = H
        self.WP = self.W + 2          # padded row: [pad, 0..W-1, pad]
        self.QH = H // 4              # rows per quarter
        assert self.QH % nr == 0
        self.nr = nr                  # output rows per quarter per slab
        self.nslab = self.QH // nr


def _f(ap):
    """Flatten free dims of a 3d tile AP to [P, fd]."""
    return ap.rearrange("p a b -> p (a b)")


def build_nc(cfg: Cfg, finalize=True):
    nc = bacc.Bacc()
    H, W, WP, nr = cfg.H, cfg.W, cfg.WP, cfg.nr
    nh = nr + 2   # OS rows:  [r0-1, r0+nr+1)
    nx = nr + 4   # x rows:   [r0-2, r0+nr+2)
    nslab = cfg.nslab

    x_in = nc.declare_dram_parameter("x", [C, H + 4, W], BF16, isOutput=False)
    woff_in = nc.declare_dram_parameter("woff", [128, 9 * OC2], BF16, isOutput=False)
    wdcn_in = nc.declare_dram_parameter("wdcn", [128, 9 * C], BF16, isOutput=False)
    boff_in = nc.declare_dram_parameter("boff", [128, 1], F32, isOutput=False)
    bdcn_in = nc.declare_dram_parameter("bdcn", [128, 1], F32, isOutput=False)
    y_out = nc.declare_dram_parameter("y", [C, H, W], F32, isOutput=True)

    with tile.TileContext(nc) as tc, ExitStack() as ctx:
        consts = ctx.enter_context(tc.tile_pool(name="consts", bufs=1))
        xpool = ctx.enter_context(tc.tile_pool(name="xp", bufs=2))
        spool = ctx.enter_context(tc.tile_pool(name="sp", bufs=1))
        sxpool = ctx.enter_context(tc.tile_pool(name="sxp", bufs=2))
        wpool = ctx.enter_context(tc.tile_pool(name="wp", bufs=1))
        dpool = ctx.enter_context(tc.tile_pool(name="dp", bufs=1))
        hpool = ctx.enter_context(tc.tile_pool(name="hp", bufs=1))
        ospool = ctx.enter_context(tc.tile_pool(name="osp", bufs=2))
        ocpool = ctx.enter_context(tc.tile_pool(name="ocp", bufs=2))
        zpool = ctx.enter_context(tc.tile_pool(name="zp", bufs=2, space="PSUM"))
        opool = ctx.enter_context(tc.tile_pool(name="op", bufs=2, space="PSUM"))

        WOFF = consts.tile([128, 9, OC2], BF16)
        nc.sync.dma_start(out=_f(WOFF), in_=woff_in[:])
        WDCN = consts.tile([128, 9, C], BF16)
        nc.sync.dma_start(out=_f(WDCN), in_=wdcn_in[:])
        BOFF = consts.tile([128, 1], F32)
        nc.sync.dma_start(out=BOFF[:], in_=boff_in[:])
        BDCN = consts.tile([128, 1], F32)
        nc.sync.dma_start(out=BDCN[:], in_=bdcn_in[:])
        PHALF = consts.tile([128, 1], F32)
        nc.vector.memset(PHALF[:], 0.5)

        # persistent x slabs: pre-zero once so pad cols stay zero forever
        XS_list = []
        for sl in range(2):
            XSz = xpool.tile([128, nx, WP], BF16, tag="xs", name=f"xsz{sl}")
            nc.vector.memset(_f(XSz), 0.0)
            XS_list.append(XSz)

        state = {}   # per-slab live tiles, keyed (name, slab)

        def zlo_of(it):
            # slab 0 computes its top halo rows; later slabs copy them from
            # the previous slab's OS tile and only compute OS rows
            # [2, nr+2) == image rows r0+1 .. r0+nr.
            return 0 if it == 0 else 2

        def load_slab(it):
            if it < 2:
                XS = XS_list[it % 2]
            else:
                XS = xpool.tile([128, nx, WP], BF16, tag="xs", name="xs")
            state["xs", it] = XS
            flo = 0 if it == 0 else 2   # steady slabs only read frame rows 2+
            for g in range(4):
                i0 = cfg.QH * g + it * nr   # row in the padded-x frame
                nc.sync.dma_start(
                    out=XS[32 * g:32 * g + 32, flo:nx, 1:W + 1],
                    in_=x_in[:, i0 + flo:i0 + nx, :])

        def stage_offset(it):
            """conv_off + sigmoid + fold + B2/Bv2 acts for slab it."""
            XS = state["xs", it]
            XSf = _f(XS[:])
            zlo = zlo_of(it)
            nz = nr + 2 - zlo
            # conv_off: pairs (q0,q1)->zts[0], (q2,q3)->zts[1]
            SP = spool.tile([128, 2, nz, WP], BF16, tag="sp", name="sp")
            for j in range(nz):
                zz = zpool.tile([128, 2, 512], F32, tag="zz", name="zz")
                for t in range(9):
                    kh, kw = t // 3, t % 3
                    base = (zlo + j + kh) * WP + kw
                    for p in range(2):
                        for gq in range(2):
                            g = 2 * p + gq
                            nc.tensor.matmul(
                                zz[64 * gq:64 * gq + 64, p, 1:W + 1],
                                lhsT=WOFF[32 * g:32 * g + 32, t, :],
                                rhs=XSf[32 * g:32 * g + 32, base:base + W],
                                start=(t == 0), stop=(t == 8),
                                tile_position=(32 * g, 64 * gq),
                                skip_group_check=True)
                nc.scalar.activation(
                    out=SP[:, :, j, 1:W + 1], in_=zz[:, :, 1:W + 1],
                    func=AF.Sigmoid, bias=BOFF[:], scale=1.0)
            # fold s into quarter-stacked SX/SY (8 sbuf->sbuf DMAs); SX first
            # so the sampling stage's A/B2 chain unblocks earliest
            SX = sxpool.tile([128, nz, WP], BF16, tag="sx")
            SY = sxpool.tile([128, nz, WP], BF16, tag="sy")
            for g in range(4):
                p, gq = g // 2, g % 2
                nc.sync.dma_start(
                    out=_f(SX[32 * g:32 * g + 32]),
                    in_=_f(SP[64 * gq:64 * gq + 32, p]))
            for g in range(4):
                p, gq = g // 2, g % 2
                nc.sync.dma_start(
                    out=_f(SY[32 * g:32 * g + 32]),
                    in_=_f(SP[64 * gq + 32:64 * gq + 64, p]))
            state["sx", it] = SX
            state["sy", it] = SY

        def stage_sample(it):
            """DL/DR + A/Av maps + horizontal interps + vertical -> OS."""
            XS = state["xs", it]
            XSf = _f(XS[:])
            zlo = zlo_of(it)
            nz = nr + 2 - zlo
            Lz = nz * WP
            # difference images (no deps beyond XS -> issued first so the
            # vector engine has work while the offset chain finishes)
            L = nx * WP
            DL = dpool.tile([128, nx, WP], BF16, tag="dl")
            DLf = _f(DL)
            nc.vector.tensor_tensor(
                DLf[:, 1:L], XSf[:, 0:L - 1], XSf[:, 1:L], OP.subtract)
            DR = dpool.tile([128, nx, WP], BF16, tag="dr")
            DRf = _f(DR)
            # DRn = x - x(c+1): sign-flipped so its weight can be the
            # vector-computable B2n = min(sx,.5)-.5 = -relu(.5-sx)
            nc.vector.tensor_tensor(
                DRf[:, 0:L - 1], XSf[:, 0:L - 1], XSf[:, 1:L], OP.subtract)
            # reflect fixups at image cols 0 / W-1 (padded cols 1 / W)
            nc.vector.tensor_tensor(
                DL[:, :, 1], DL[:, :, 1], XS[:, :, 2], OP.add)
            nc.vector.tensor_tensor(
                DR[:, :, W], DR[:, :, W], XS[:, :, W - 1], OP.subtract)
            # copy the 2 shared halo rows of OS from the previous slab
            OS = ospool.tile([128, nh, WP], BF16, tag="os")
            if it > 0:
                OSprev = state["os", it - 1]
                nc.vector.tensor_copy(
                    _f(OS[:, 0:2, :]), _f(OSprev[:, nr:nr + 2, :]))
                del state["os", it - 1]
            # weight maps, all on DVE (4x TS):
            #   B2n = min(sx,.5)-.5 = -relu(.5-sx)   (pairs with DRn)
            #   A   = max(sx,.5)-.5 =  relu(sx-.5)   (in place over SX,
            #         emitted after B2n so the read precedes the overwrite)
            SX, SY = state["sx", it], state["sy", it]
            B2 = wpool.tile([128, nz, WP], BF16, tag="b2n")
            nc.vector.tensor_scalar(_f(B2), _f(SX), 0.5, 0.5, OP.min,
                                    OP.subtract)
            nc.vector.tensor_scalar(_f(SX), _f(SX), 0.5, 0.5, OP.max,
                                    OP.subtract)
            A = SX

            # ---- horizontal interps for all three dr in one op each ----
            # plane k of the [128,3,Lz] APs is row-shift dr=k-1; weights are
            # broadcast (stride-0 plane dim).
            def sh3(flat, n):
                b = flat[:, zlo * WP:zlo * WP + Lz]
                lay = [list(b.ap[0]), [WP, n], [1, Lz]]
                return bass.AP(b.tensor, b.offset, lay)

            X3 = sh3(XSf, 3)
            DL3 = sh3(DLf, 3)
            DR3 = sh3(DRf, 3)
            A3 = _f(A).unsqueeze(1).broadcast_to((128, 3, Lz))
            B3 = _f(B2).unsqueeze(1).broadcast_to((128, 3, Lz))
            T1 = hpool.tile([128, 3, nz, WP], BF16, tag="ht1", name="t1all")
            T1c = T1.rearrange("p a b c -> p a (b c)")
            nc.vector.tensor_tensor(T1c, A3, DL3, OP.mult)
            T2 = hpool.tile([128, 3, nz, WP], BF16, tag="ht2", name="t2all")
            T2c = T2.rearrange("p a b c -> p a (b c)")
            nc.vector.tensor_tensor(T2c, B3, DR3, OP.mult)
            nc.vector.tensor_tensor(T1c, T1c, X3, OP.add)
            nc.vector.tensor_tensor(T1c, T1c, T2c, OP.add)
            Hall = T1c          # planes: (Hm, H0, Hp)

            # ---- vertical: OS = H0 + Av*(Hm-H0) + Bv2n*(H0-Hp) ----
            BV2 = wpool.tile([128, 2, nz, WP], BF16, tag="avb")
            BVc = BV2.rearrange("p a b c -> p a (b c)")
            nc.vector.tensor_scalar(BVc[:, 1], _f(SY), 0.5, 0.5, OP.min,
                                    OP.subtract)
            nc.vector.tensor_scalar(BVc[:, 0], _f(SY), 0.5, 0.5, OP.max,
                                    OP.subtract)
            DH = hpool.tile([128, 2, nz, WP], BF16, tag="ht2", name="dh")
            DHc = DH.rearrange("p a b c -> p a (b c)")
            # plane0 = Hm-H0, plane1 = H0-Hp
            nc.vector.tensor_tensor(DHc, Hall[:, 0:2], Hall[:, 1:3],
                                    OP.subtract)
            # T34 in place over DH: plane0 = Av*(Hm-H0), plane1 = Bv2n*(H0-Hp)
            nc.vector.tensor_tensor(DHc, DHc, BVc, OP.mult)
            # reflect fixups at image rows 0 / H-1: recompute the swapped
            # term from the intact Hall planes
            if it == 0:
                nc.vector.tensor_tensor(
                    DHc[0:32, 0, WP:2 * WP], Hall[0:32, 2, WP:2 * WP],
                    Hall[0:32, 1, WP:2 * WP], OP.subtract)
                nc.vector.tensor_tensor(
                    DHc[0:32, 0, WP:2 * WP], DHc[0:32, 0, WP:2 * WP],
                    BVc[0:32, 0, WP:2 * WP], OP.mult)
            if it == nslab - 1:
                jb = nr - zlo   # OS row nr == image row H-1 (quarter 3)
                sl = slice(jb * WP, (jb + 1) * WP)
                nc.vector.tensor_tensor(
                    DHc[96:128, 1, sl], Hall[96:128, 1, sl],
                    Hall[96:128, 0, sl], OP.subtract)
                nc.vector.tensor_tensor(
                    DHc[96:128, 1, sl], DHc[96:128, 1, sl],
                    BVc[96:128, 1, sl], OP.mult)
            # final adds in two row-halves, with the output-conv chunk for
            # each half issued as soon as its OS rows are ready
            state["os", it] = OS
            zmid = 6        # OS rows [0,6) feed out rows 0..3, rest 4..7
            for half, (ja, jb2) in enumerate(((zlo, zmid), (zmid, nr + 2))):
                sl = slice((ja - zlo) * WP, (jb2 - zlo) * WP)
                OSc = _f(OS[:, ja:jb2, :])
                nc.vector.tensor_tensor(OSc, Hall[:, 1, sl], DHc[:, 0, sl],
                                        OP.add)
                nc.vector.tensor_tensor(OSc, OSc, DHc[:, 1, sl], OP.add)
                # sampled outside the image is 0 for final-conv zero-padding
                nc.vector.memset(OS[:, ja:jb2, 0:WP:W + 1], 0.0)
                if half == 0:
                    if it > 0:
                        nc.vector.memset(OS[:, 0:2, 0:WP:W + 1], 0.0)
                    if it == 0:
                        nc.vector.memset(_f(OS[0:32, 0:1, :]), 0.0)
                else:
                    if it == nslab - 1:
                        nc.vector.memset(_f(OS[96:128, nr + 1:nr + 2, :]), 0.0)
                outconv_chunk(it, half)

        def outconv_chunk(it, oc_i):
            """conv_dcn + bias + store for one 4-row output chunk."""
            OSf = _f(state["os", it])
            r0 = it * nr
            OROWS = 4
            if True:
                OC_t = ocpool.tile([128, OROWS, WP], F32, tag="oc")
                for oj in range(OROWS):
                    oi = oc_i * OROWS + oj
                    ot = opool.tile([128, 512], F32, tag="ot")
                    for t in range(9):
                        kh, kw = t // 3, t % 3
                        base = (oi + kh) * WP + kw
                        for g in range(4):
                            nc.tensor.matmul(
                                ot[32 * g:32 * g + 32, 1:W + 1],
                                lhsT=WDCN[32 * g:32 * g + 32, t, :],
                                rhs=OSf[32 * g:32 * g + 32, base:base + W],
                                start=(t == 0), stop=(t == 8),
                                tile_position=(32 * g, 32 * g),
                                skip_group_check=True)
                    nc.scalar.activation(
                        out=OC_t[:, oj, 1:W + 1], in_=ot[:, 1:W + 1],
                        func=AF.Identity, bias=BDCN[:], scale=1.0)
                for g in range(4):
                    rr = cfg.QH * g + r0 + oc_i * OROWS
                    nc.sync.dma_start(
                        out=y_out[:, rr:rr + OROWS, :],
                        in_=OC_t[32 * g:32 * g + 32, :, 1:W + 1])

        # ---- software-pipelined slab loop ----
        # emission order per iteration: offset(i) | sample(i-1), outconv(i-1)
        # | load(i+1), so the PE's conv_off(i) overlaps the vector work of
        # sample(i-1).
        load_slab(0)
        for it in range(nslab + 1):
            if it < nslab:
                stage_offset(it)
            if it > 0:
                stage_sample(it - 1)
            if it + 1 < nslab:
                load_slab(it + 1)
    if finalize:
        nc.finalize()
    return nc


def prep_weights(w_off, b_off, w_dcn, b_dcn):
    """Host-side packing of conv weights into lhsT tiles, replicated x4."""
    perm = np.concatenate([np.arange(0, 2 * C, 2), np.arange(1, 2 * C, 2)])
    # WOFF[32g+ci, kh*3+kw, m] = w_off[perm[m], ci, kh, kw]
    wo = w_off[perm].astype(np.float32)            # [64, C, 3, 3]
    wo = wo.transpose(1, 2, 3, 0).reshape(C, 9, OC2)   # [ci, tap, m]
    woff = np.tile(wo, (4, 1, 1)).reshape(128, 9 * OC2)
    wd = w_dcn.astype(np.float32).transpose(1, 2, 3, 0).reshape(C, 9, C)
    wdcn = np.tile(wd, (4, 1, 1)).reshape(128, 9 * C)
    boff = np.tile(b_off[perm].astype(np.float32), 2).reshape(128, 1)
    bdcn = np.tile(b_dcn.astype(np.float32), 4).reshape(128, 1)
    return {
        "woff": woff.astype(ml_dtypes.bfloat16),
        "wdcn": wdcn.astype(ml_dtypes.bfloat16),
        "boff": boff.astype(np.float32),
        "bdcn": bdcn.astype(np.float32),
    }


_NC_CACHE = {}


def _get_nc(cfg_key):
    if cfg_key not in _NC_CACHE:
        _NC_CACHE[cfg_key] = build_nc(Cfg(H=cfg_key[0], nr=cfg_key[1]))
    return _NC_CACHE[cfg_key]


def _run(x, w_off, b_off, w_dcn, b_dcn, **spmd_kwargs):
    from concourse.bass_utils import run_bass_kernel_spmd

    B = x.shape[0]
    H = x.shape[2]
    assert x.shape == (B, C, H, H) and B == N_CORES
    nc = _get_nc((H, 8))
    w = prep_weights(np.asarray(w_off), np.asarray(b_off),
                     np.asarray(w_dcn), np.asarray(b_dcn))
    in_maps = []
    for b in range(B):
        m = dict(w)
        xb = np.asarray(x[b]).astype(ml_dtypes.bfloat16)
        m["x"] = np.pad(xb, ((0, 0), (2, 2), (0, 0)))
        in_maps.append(m)
    return run_bass_kernel_spmd(nc, in_maps, list(range(N_CORES)), **spmd_kwargs)


def kernel(x, w_off, b_off, w_dcn, b_dcn):
    res = _run(x, w_off, b_off, w_dcn, b_dcn)
    out = np.stack([res.results[i]["y"] for i in range(N_CORES)], axis=0)
    return out.astype(np.float32)
